# revision 2
# baseline (speedup 1.0000x reference)
"""Trainium2 Bass kernel for nn_DeltaRule (gated two-channel linear-attention scan).

v3 design (vs the v1 baseline):
  * q/k/v shipped to HBM as bf16 (host cast); q,k HOST-TRANSPOSED to [D,T] so
    the S' matmuls need no on-chip transposes; output returned bf16->fp32 on
    host.  HBM traffic per core drops 16.8MB -> ~10MB.
  * Two decay channels merged via the Toeplitz identity
        D2[s,t] = D1[s,t] * rho^(t-s),  rho = b2c/b1c   (exact when unclipped;
    clipping only matters for beta<0.0114 where the decay is ~0.01/step and
    the affected contributions are 100x suppressed)
    so only ONE log-decay matrix is exponentiated; (1 + rho^dt) (+ causal
    mask) is a constant bf16 matrix multiplied into the attention weights.
  * phi(x) = elu(x)+1 computed as min(exp(x), x+1)  (exact identity: e^x>=x+1
    with equality branch switching at 0; |x|<~6 here so exp never overflows).
  * The L-row broadcast (L1[c,t] replicated to 128 partitions) is done by a
    DMA from a DRAM scratch with a 0-stride partition AP - no compute engine.
  * Decay pipe batched per slab: 8 tensor_scalar arg builds -> ONE exp ->
    ONE mask/rho multiply over [128, 1024].
  * S' matmuls write bf16 PSUM, paired two chunks per tile so the A = S'*D
    multiply runs at DVE 2x over [128, 512].
  * den via N=1 ones-matmuls on PE; normalization on the Pool engine.

Math (reference):
    phi(x) = elu(x)+1;  b_in = clip(beta, .01, .995)
    b1_t = clip(sigmoid(2)*b_in, .01, .995);  b2_t analogous with sigmoid(3)
    H_ch(t) = sum_{s<=t} (prod_{j=s+1..t} b_ch,j) phi_k(s) v_s^T
    o_t = [phi_q(t).(H1+H2)] / max(phi_q(t).(Z1+Z2), 1e-6)
Decay products vanish (<4.5e-8) beyond 128 steps (b1<=0.8766), so each
128-step output chunk attends over a 256-step window (prev chunk + itself)
with exact decay weights exp(L_t - L_s); cross-chunk stitching uses
rev[s] = L_prev[end] - L_prev[s].  All (batch, chunk) tasks independent;
batch dim (16) shards across the 8 NeuronCores (2 per core).
"""

import math

import numpy as np
import ml_dtypes

import concourse.bass as bass
import concourse.tile as tile
import concourse.mybir as mybir
import concourse.bass_utils as bass_utils

F32 = mybir.dt.float32
BF16 = mybir.dt.bfloat16
F16 = mybir.dt.float16
AF = mybir.ActivationFunctionType
ALU = mybir.AluOpType

B, T, D = 16, 4096, 128
C = 128                 # chunk length
NCHUNK = T // C         # 32
SLAB = 4                # chunks per DMA slab
NCORES = 8
BPC = B // NCORES       # batches per core
BETA_MIN, BETA_MAX, EPS = 0.01, 0.995, 1e-6
NEG_BIG = -1.0e9


def _split_multi_waits(nc):
    """This container's walrus supports only ONE sync-wait command per
    instruction; Tile attaches several.  Split extras onto preceding
    same-engine nops (engines are in-order, so semantics are unchanged)."""
    for fn in nc.m.functions:
        for bb in fn.blocks:
            new = []
            for ins in bb.instructions:
                si = getattr(ins, "sync_info", None)
                ow = list(si.on_wait) if (si is not None and si.on_wait) else []
                if len(ow) > 1:
                    for j, w in enumerate(ow[:-1]):
                        nop = mybir.InstNoOp(name=f"{ins.name}_ws{j}", ins=[], outs=[])
                        nop.engine = ins.engine
                        nop.sync_info = mybir.SyncInfo(on_wait=[w], on_update=[])
                        new.append(nop)
                    si.on_wait = [ow[-1]]
                ou = list(si.on_update) if (si is not None and si.on_update) else []
                if len(ou) > 1 and type(ins).__name__ != "InstDMACopy":
                    new.append(ins)
                    for j, u in enumerate(ou[1:]):
                        nop = mybir.InstNoOp(name=f"{ins.name}_us{j}", ins=[], outs=[])
                        nop.engine = ins.engine
                        nop.sync_info = mybir.SyncInfo(on_wait=[], on_update=[u])
                        new.append(nop)
                    si.on_update = [ou[0]]
                    continue
                new.append(ins)
            bb.instructions = new


def _build_kernel(nc, b1c: float, b2c: float):
    NSLAB = NCHUNK // SLAB
    # qkv packed per (batch, slab): [128, 1536] bf16 = [qk(+1) d-major | v token-major]
    qkv_d = nc.dram_tensor("qkv", [BPC, NSLAB, 128, 3 * SLAB * C], F16,
                           kind="ExternalInput").ap()
    lh_d = nc.dram_tensor("lh", [BPC, NCHUNK, C], F32, kind="ExternalInput").ap()
    ch_d = nc.dram_tensor("colsh", [BPC, 128, 2 * NCHUNK], F32,
                          kind="ExternalInput").ap()
    mc_d = nc.dram_tensor("mconst", [128, SLAB * 2 * C], F16, kind="ExternalInput").ap()
    o_d = nc.dram_tensor("o", [BPC, NSLAB, 128, SLAB * C], F16,
                         kind="ExternalOutput").ap()

    with tile.TileContext(nc) as tc:
        with (
            tc.tile_pool(name="const", bufs=1) as cpool,
            tc.tile_pool(name="bmeta", bufs=2) as bmp,
            tc.tile_pool(name="slab", bufs=8) as slp,
            tc.tile_pool(name="work", bufs=8) as wp,
            tc.tile_pool(name="ps_s", bufs=4, space="PSUM") as ps_s,
            tc.tile_pool(name="ps_o", bufs=2, space="PSUM") as ps_o,
            tc.tile_pool(name="ps_d", bufs=2, space="PSUM") as ps_d,
        ):
            mconst = cpool.tile([128, SLAB * 2 * C], F16)
            nc.sync.dma_start(mconst[:], mc_d[:])
            ones = cpool.tile([128, 1], F16)
            nc.gpsimd.memset(ones[:], 1.0)
            neg1 = cpool.tile([128, 1], F32)
            nc.gpsimd.memset(neg1[:], -1.0)

            batch_cols = []
            for b in range(BPC):
                cols = bmp.tile([128, 2 * NCHUNK], F32, tag="cols")
                nc.sync.dma_start(cols[:], ch_d[b])
                batch_cols.append(cols)

            prevs = [None] * BPC
            a2ctr = [0]

            def stage_a(i):
                sb, b = divmod(i, BPC)
                c0 = sb * SLAB
                cols = batch_cols[b]
                st = {"sb": sb, "b": b, "c0": c0}
                # L1 rows of the slab chunks replicated to all partitions
                lbs = slp.tile([128, SLAB * C], F32, tag="lbs")
                nc.sync.dma_start(
                    lbs[:].rearrange("p (n d) -> p n d", d=C),
                    lh_d[b, c0 : c0 + SLAB, :].partition_broadcast(128),
                )
                qkvs = slp.tile([128, 3 * SLAB * C], F16, tag="qkvs")
                nc.sync.dma_start(qkvs[:], qkv_d[b, sb])
                st["qks"] = qkvs[:, 0 : 2 * SLAB * C]
                st["vs"] = qkvs[:, 2 * SLAB * C :]
                # decay args -> exp -> mask/rho multiply, per pair of chunks
                dms = []
                for pr in range(SLAB // 2):
                    argt = wp.tile([128, 2 * 2 * C], F32, tag=f"argt{pr}")
                    for j in range(2):
                        cs = 2 * pr + j
                        c = c0 + cs
                        o0 = j * 2 * C
                        if c == 0:
                            nc.gpsimd.memset(argt[:, o0 : o0 + C], NEG_BIG)
                        else:
                            nc.gpsimd.tensor_scalar(
                                argt[:, o0 : o0 + C],
                                lbs[:, cs * C : (cs + 1) * C],
                                cols[:, c - 1 : c], 0.0, ALU.add, ALU.min,
                            )
                        nc.gpsimd.tensor_scalar(
                            argt[:, o0 + C : o0 + 2 * C],
                            lbs[:, cs * C : (cs + 1) * C],
                            cols[:, NCHUNK + c : NCHUNK + c + 1],
                            0.0, ALU.add, ALU.min,
                        )
                    d1 = wp.tile([128, 2 * 2 * C], F16, tag=f"d1{pr}")
                    nc.scalar.activation(d1[:], argt[:], AF.Exp)
                    dm = wp.tile([128, 2 * 2 * C], F16, tag=f"dm{pr}")
                    dmeng = nc.vector
                    dmeng.tensor_tensor(
                        dm[:], d1[:], mconst[:, pr * 4 * C : (pr + 1) * 4 * C],
                        ALU.mult,
                    )
                    dms.append(dm)
                st["dms"] = dms
                # phi = min(exp(x), x+1); host ships qk PRE-INCREMENTED (x+1)
                et = slp.tile([128, 2 * SLAB * C], F16, tag="et")
                nc.scalar.activation(et[:], st["qks"], AF.Exp, bias=neg1[:])
                u = slp.tile([128, 2 * SLAB * C], F16, tag="u")
                nc.vector.tensor_scalar(u[:], st["qks"], 1.0, None, ALU.max)
                phis = slp.tile([128, 2 * SLAB * C], F16, tag="phis")
                nc.vector.tensor_tensor(phis[:], u[:], et[:], ALU.min)
                st["phis"] = phis
                return st

            def stage_b(st):
                sb, b, c0 = st["sb"], st["b"], st["c0"]
                phis, vs, dms = st["phis"], st["vs"], st["dms"]
                pso_slab = ps_o.tile([128, SLAB * C], F32, tag="pso")
                psden = ps_d.tile([128, SLAB], F32, tag="psden")
                st["pso"] = pso_slab
                st["psden"] = psden
                for pr in range(SLAB // 2):
                    pss = ps_s.tile([128, 2 * 2 * C], F32, tag="pss")
                    pair_chunks = (c0 + 2 * pr, c0 + 2 * pr + 1)
                    phik_v = []
                    for j, c in enumerate(pair_chunks):
                        cs = 2 * pr + j
                        phiq = phis[:, cs * C : (cs + 1) * C]
                        phik = phis[:, (SLAB + cs) * C : (SLAB + cs + 1) * C]
                        vcur = vs[:, cs * C : (cs + 1) * C]
                        prev = prevs[b]
                        pk_prev, v_prev = prev if prev is not None else (phik, vcur)
                        o0 = j * 2 * C
                        nc.tensor.matmul(
                            pss[:, o0 : o0 + C], pk_prev, phiq, start=True, stop=True
                        )
                        nc.tensor.matmul(
                            pss[:, o0 + C : o0 + 2 * C], phik, phiq,
                            start=True, stop=True,
                        )
                        phik_v.append((pk_prev, v_prev, vcur))
                        prevs[b] = (phik, vcur)
                    a2 = wp.tile([128, 2 * 2 * C], F16, tag="a2")
                    nc.vector.tensor_tensor(a2[:], pss[:], dms[pr][:], ALU.mult)
                    for j, c in enumerate(pair_chunks):
                        cs = 2 * pr + j
                        _, v_prev, vcur = phik_v[j]
                        o0 = j * 2 * C
                        pso = pso_slab[:, cs * C : (cs + 1) * C]
                        nc.tensor.matmul(
                            pso, a2[:, o0 : o0 + C], v_prev, start=True, stop=False
                        )
                        nc.tensor.matmul(
                            pso, a2[:, o0 + C : o0 + 2 * C], vcur,
                            start=False, stop=True,
                        )
                        nc.tensor.matmul(
                            psden[:, cs : cs + 1], a2[:, o0 : o0 + C], ones[:],
                            start=True, stop=False,
                        )
                        nc.tensor.matmul(
                            psden[:, cs : cs + 1], a2[:, o0 + C : o0 + 2 * C],
                            ones[:], start=False, stop=True,
                        )
            def stage_c(st):
                sb, b = st["sb"], st["b"]
                pso_slab, psden = st["pso"], st["psden"]
                ots = slp.tile([128, SLAB * C], F16, tag="ots")
                den4 = wp.tile([128, SLAB], F32, tag="den4")
                nc.vector.tensor_scalar(den4[:], psden[:], EPS, None, ALU.max)
                rden4 = wp.tile([128, SLAB], F32, tag="rden4")
                nc.vector.reciprocal(rden4[:], den4[:])
                for j in range(SLAB):
                    if j == 0:
                        nc.vector.tensor_scalar(
                            ots[:, j * C : (j + 1) * C],
                            pso_slab[:, j * C : (j + 1) * C],
                            rden4[:, j : j + 1], None, ALU.mult,
                        )
                    else:
                        nc.scalar.activation(
                            ots[:, j * C : (j + 1) * C],
                            pso_slab[:, j * C : (j + 1) * C],
                            AF.Copy, scale=rden4[:, j : j + 1],
                        )
                nc.scalar.dma_start(o_d[b, sb], ots[:])

            NIT = NSLAB * BPC
            sts = [None] * NIT
            sts[0] = stage_a(0)
            sts[1] = stage_a(1)
            for i in range(NIT):
                if i + 2 < NIT:
                    sts[i + 2] = stage_a(i + 2)
                stage_b(sts[i])
                if i - 1 >= 0:
                    stage_c(sts[i - 1])
                    sts[i - 1] = None
            stage_c(sts[NIT - 1])
    return nc


def _host_prep(q, k, v, beta, base_beta_1, base_beta_2):
    q = np.asarray(q, dtype=np.float32)
    k = np.asarray(k, dtype=np.float32)
    v = np.asarray(v, dtype=np.float32)
    beta = np.asarray(beta, dtype=np.float32).reshape(B, NCHUNK, C)
    bb1 = float(np.asarray(base_beta_1))
    bb2 = float(np.asarray(base_beta_2))
    b1c = float(np.clip(1.0 / (1.0 + math.exp(-bb1)), BETA_MIN, BETA_MAX))
    b2c = float(np.clip(1.0 / (1.0 + math.exp(-bb2)), BETA_MIN, BETA_MAX))
    rho = b2c / b1c
    NSLAB = NCHUNK // SLAB

    # qkv packed per (batch, slab): [qk(+1) d-major | v token-major]
    qt = (q.transpose(0, 2, 1) + 1.0).astype(np.float16)   # [B, D, T]
    kt = (k.transpose(0, 2, 1) + 1.0).astype(np.float16)
    qk = np.concatenate(
        [qt.reshape(B, D, NSLAB, SLAB * C), kt.reshape(B, D, NSLAB, SLAB * C)],
        axis=3,
    ).transpose(0, 2, 1, 3)                                        # [B, NSLAB, 128, 1024]
    vp = (
        v.astype(np.float16)
        .reshape(B, NSLAB, SLAB, 128, D)
        .transpose(0, 1, 3, 2, 4)
        .reshape(B, NSLAB, 128, SLAB * C)
    )
    qkv = np.concatenate([qk, vp], axis=3)                # [B, NSLAB, 128, 1536]

    # decay metadata (host): L1 = per-chunk cumsum of log(clip(b1c*b_in))
    b_in = np.clip(beta, BETA_MIN, BETA_MAX)
    g1 = np.maximum(b1c * b_in, BETA_MIN)
    L1 = np.cumsum(np.log(g1), axis=2, dtype=np.float64).astype(np.float32)
    revlog = L1[:, :, C - 1 : C] - L1                              # [B, NCHUNK, C], <= 0
    colsh = np.concatenate(
        [revlog.transpose(0, 2, 1), -L1.transpose(0, 2, 1)], axis=2
    ).astype(np.float32)                                           # [B, 128, 2*NCHUNK]

    # mconst[s, t']: prev half gets 1 + rho^(C+t-s); cur half causal mask with
    # 1 + rho^(t-s).  Tiled SLAB times for the per-pair multiplies.
    sidx = np.arange(C)[:, None]
    tidx = np.arange(C)[None, :]
    mprev = np.minimum(1.0 + rho ** (C + tidx - sidx), 60000.0)
    mcur = np.minimum(1.0 + rho ** (tidx - sidx), 60000.0) * (tidx >= sidx)
    mchunk = np.concatenate([mprev, mcur], axis=1)
    mconst = np.tile(mchunk, (1, SLAB)).astype(np.float16)

    in_maps = []
    for i in range(NCORES):
        sl = slice(i * BPC, (i + 1) * BPC)
        in_maps.append(
            {
                "qkv": np.ascontiguousarray(qkv[sl]),
                "lh": np.ascontiguousarray(L1[sl]),
                "colsh": np.ascontiguousarray(colsh[sl]),
                "mconst": mconst,
            }
        )
    return in_maps, b1c, b2c


def build_nc(b1c: float, b2c: float):
    nc = bass.Bass("TRN2", target_bir_lowering=False, debug=False, num_devices=NCORES)
    _build_kernel(nc, b1c, b2c)
    _split_multi_waits(nc)
    return nc


def kernel(q, k, v, beta, mask, base_beta_1, base_beta_2):
    in_maps, b1c, b2c = _host_prep(q, k, v, beta, base_beta_1, base_beta_2)
    nc = build_nc(b1c, b2c)
    res = bass_utils.run_bass_kernel_spmd(nc, in_maps, core_ids=list(range(NCORES)))
    out = np.empty((B, T, D), dtype=np.float32)
    NSLAB = NCHUNK // SLAB
    for i in range(NCORES):
        op = res.results[i]["o"].astype(np.float32)
        op = (
            op.reshape(BPC, NSLAB, 128, SLAB, D)
            .transpose(0, 1, 3, 2, 4)
            .reshape(BPC, T, D)
        )
        out[i * BPC : (i + 1) * BPC] = op
    return out
